# revision 65
# baseline (speedup 1.0000x reference)
"""Causal GQA attention with RoPE for Trainium2, sharded over 8 NeuronCores.

Problem: x[4,1024,2048] @ wq/wk/wv -> RoPE -> causal GQA attention -> @ wo.
H=32 q-heads, KVH=8 kv-heads (GQA rep 4), D=64.

Sharding: core = 2*b + g  (b = batch 0..3, g = head-group 0..1).
Each core handles one batch and 16 q-heads / 4 kv-heads, computing a partial
output projection; the host sums the two head-group partials per batch.

v2 design notes (all timings per the TRN2 cost model):
  - every matmul operand is bfloat16: full PE rate at any moving width
    (fp32r drops to 1/4 rate below 256 cols), half the DMA traffic, and
    2x DVE throughput on 16-bit elementwise ops.  PSUM accumulation stays
    fp32; softmax denominators and the output stay fp32.
  - all inputs are pre-packed on the host into the exact SBUF tile layout,
    so every DMA is a plain contiguous [128, N] copy with >=512B runs
    (full DMA rate) and the arrival order can be scheduled precisely.
  - rope: one Act psum->bf16 copy, then the pair-swap+sin multiply is
    fused into four cross-partition-base tensor_muls on DVE (sin tile is
    indexed by the swap SOURCE row so both SBUF inputs share a base
    partition); finally dest += straight*cos.  The Activation engine is
    left almost exclusively to softmax exp, the attention-phase rate
    limiter (Pool cannot read PSUM, so exp and all psum readers live on
    Act/DVE only).
  - softmax row-sums come from 64 ones-columns appended to v: the attn@v
    matmul lands sum(E) replicated on psum partitions 64..127, so
    normalization is one DVE reciprocal + one DVE multiply (no gpsimd
    partition_broadcast on the critical path).
  - causal masking at 128-key granularity (36 of 64 blocks per head, the
    optimum); diagonal blocks are emitted FIRST within each accumulation
    and their post-exp mask multiplies alternate Pool/DVE.
  - attention is software-pipelined: the attn@v pair for step s-1 issues
    while step s's scores run, and the next q-chunk's projection chain is
    drip-fed between steps from a filler queue; the tensor engine's clock
    ramp is pre-warmed by a dummy matmul during the DMA lead-in, and the
    first out-projection chains open (hd 0..6) inside the last attention
    block, which has no projection fillers left.
"""

import os

import numpy as np

import concourse.bacc as bacc
import concourse.bass as bass
import concourse.mybir as mybir
import concourse.tile as tile
from concourse.bass_utils import run_bass_kernel_spmd

B, S, DIM = 4, 1024, 2048
H, KVH, D = 32, 8, 64
HL = H // 2        # 16 q heads per core
KVL = KVH // 2     # 4 kv heads per core
QCOLS = HL * D     # 1024
KCOLS = KVL * D    # 256
NB = 512           # matmul moving-dim block (one PSUM bank of fp32)
P = 128
KC = DIM // P      # 16 contraction chunks

F32 = mybir.dt.float32
BF = mybir.dt.bfloat16
Exp = mybir.ActivationFunctionType.Exp
MULT = mybir.AluOpType.mult


def build_program():
    nc = bacc.Bacc()

    # host-prepacked inputs: each is already in SBUF tile layout, so DMAs
    # are contiguous [128, N] row copies at full DMA rate.
    xH = nc.dram_tensor("xH", [P, 2 * KC * NB], BF, kind="ExternalInput")
    wkH = nc.dram_tensor("wkH", [P, 2 * KC * P], BF, kind="ExternalInput")
    wqH = nc.dram_tensor("wqH", [P, 4 * KC * 256], BF, kind="ExternalInput")
    wvH = nc.dram_tensor("wvH", [P, KC * KCOLS], BF, kind="ExternalInput")
    woH = nc.dram_tensor("woH", [P, 8 * 8 * 256], BF, kind="ExternalInput")
    cosP = nc.dram_tensor("cosP", [P, S], BF, kind="ExternalInput")
    # sin indexed by SOURCE row of the pair-swap (tensor_tensor requires
    # equal base partitions for its two SBUF inputs; the output may shift)
    sinSP = nc.dram_tensor("sinSP", [P, S], BF, kind="ExternalInput")
    maskP = nc.dram_tensor("maskP", [P, P], BF, kind="ExternalInput")
    outT = nc.dram_tensor("outT", [DIM, S], F32, kind="ExternalOutput")

    with tile.TileContext(nc) as tc:
        from contextlib import ExitStack
        es = ExitStack()
        with es:
            const = es.enter_context(tc.tile_pool(name="const", bufs=1))
            xtp = es.enter_context(tc.tile_pool(name="xtp", bufs=1))
            wkp = es.enter_context(tc.tile_pool(name="wkp", bufs=1))
            wvrp = es.enter_context(tc.tile_pool(name="wvrp", bufs=1))
            wstp = es.enter_context(tc.tile_pool(name="wstp", bufs=3))
            wop = es.enter_context(tc.tile_pool(name="wop", bufs=3))
            kdupp = es.enter_context(tc.tile_pool(name="kdupp", bufs=1))
            vaugp = es.enter_context(tc.tile_pool(name="vaugp", bufs=1))
            aotp = es.enter_context(tc.tile_pool(name="aotp", bufs=1))
            qrtp = es.enter_context(tc.tile_pool(name="qrtp", bufs=3))
            spool = es.enter_context(tc.tile_pool(name="spool", bufs=3))
            epool = es.enter_context(tc.tile_pool(name="epool", bufs=5))
            rpool = es.enter_context(tc.tile_pool(name="rpool", bufs=2))
            outp = es.enter_context(tc.tile_pool(name="outp", bufs=3))
            psum_mm = es.enter_context(
                tc.tile_pool(name="psum_mm", bufs=3, space="PSUM"))
            psum_oa = es.enter_context(
                tc.tile_pool(name="psum_oa", bufs=2, space="PSUM"))
            psum_sc = es.enter_context(
                tc.tile_pool(name="psum_sc", bufs=3, space="PSUM"))

            # ---- persistent tiles ----
            cost = const.tile([P, S], BF, name="cost")
            sintS = const.tile([P, S], BF, name="sintS")
            maskt = const.tile([P, P], BF, name="maskt")
            kdup = [kdupp.tile([P, S], BF, name=f"kdup{i}") for i in range(KVL)]
            # v with 64 ones-columns: attn@v then produces the softmax
            # denominator replicated on psum partitions 64..127.
            vaug = [[vaugp.tile([P, D + 64], BF, name=f"vaug{kv}_{ic}")
                     for ic in range(S // P)] for kv in range(KVL)]
            aot = [aotp.tile([P, S], BF, name=f"aot{j}") for j in range(8)]

            # x tiles: xt[ib][g] holds chunks 4g..4g+3, columns ib*512..+512
            xt_half = [[xtp.tile([P, 4 * NB], BF, name=f"xt{ib}_{g}")
                        for g in range(4)] for ib in range(2)]

            def xchunk(c, ib):       # [P, NB] view of x chunk c, col block ib
                g, cc = c // 4, c % 4
                return xt_half[ib][g][:, cc * NB:(cc + 1) * NB]

            wkg = [wkp.tile([P, KC * P], BF, name=f"wkg{jk}")
                   for jk in range(2)]
            wvall = wvrp.tile([P, KC * KCOLS], BF, name="wvall")
            wvt = [wvall[:, c * KCOLS:(c + 1) * KCOLS] for c in range(KC)]

            # ---- PE clock warmup ----
            # The tensor engine ramps 0.65->1.2->2.4 GHz over the 3us after
            # its first instruction.  Fire a trivial matmul immediately so
            # the ramp elapses during the DMA lead-in.
            warm = const.tile([1, 4], BF, name="warm")
            nc.gpsimd.memset(warm[:], 0.0)
            wps = psum_sc.tile([1, 4], F32, tag="sc", name="wps")
            nc.tensor.matmul(wps[:], warm[:, 0:1], warm[:],
                             start=True, stop=True)

            # ---- DMA issue order (single serial queue; first-needed first)
            # the first x half-tile and wk chunk are split so the first
            # K-projection matmul can issue as early as possible
            for cc in range(4):
                nc.sync.dma_start(xt_half[0][0][:, cc * NB:(cc + 1) * NB],
                                  xH[:, cc * NB:(cc + 1) * NB])
                if cc == 0:
                    nc.sync.dma_start(wkg[0][:, 0:8 * P],
                                      wkH[:, 0:8 * P])
                elif cc == 1:
                    nc.sync.dma_start(wkg[0][:, 8 * P:KC * P],
                                      wkH[:, 8 * P:KC * P])
            nc.sync.dma_start(wkg[1][:], wkH[:, KC * P:2 * KC * P])
            for g in range(1, 4):
                nc.sync.dma_start(xt_half[0][g][:],
                                  xH[:, g * 4 * NB:(g + 1) * 4 * NB])
            nc.sync.dma_start(cost[:], cosP[:])
            nc.sync.dma_start(sintS[:], sinSP[:])
            nc.sync.dma_start(wvall[:], wvH[:])
            base1 = 4 * NB * 4
            for g in range(4):
                nc.sync.dma_start(
                    xt_half[1][g][:],
                    xH[:, base1 + g * 4 * NB:base1 + (g + 1) * 4 * NB])

            def load_wq_pair(pair):
                wqg = wstp.tile([P, KC * 256], BF, tag="wqpair")
                nc.sync.dma_start(
                    wqg[:], wqH[:, pair * KC * 256:(pair + 1) * KC * 256])
                return wqg

            wq_pair0 = load_wq_pair(0)
            wq_tiles = {0: wq_pair0}
            nc.sync.dma_start(maskt[:], maskP[:])

            # ones-columns of vaug (constant, disjoint from the v writes)
            for kv in range(KVL):
                for ic in range(S // P):
                    nc.gpsimd.memset(vaug[kv][ic][:, D:D + 64], 1.0)

            # ---- helpers ----
            def rope(ps, ib, dst, dsl):
                """psum [128, NB] fp32 -> roped bf16 into dst[:, dsl].

                The pair-swap is fused into four cross-partition-base
                tensor_muls (straight[t1-half] * sin -> dest[t0-half] etc.,
                split DVE/Pool); then dest += straight * cos."""
                sl = slice(ib * NB, (ib + 1) * NB)
                straight = spool.tile([P, NB], BF, tag="straight")
                nc.scalar.copy(straight[:], ps[:])
                nc.vector.tensor_mul(
                    dst[0:32, dsl], straight[32:64, :], sintS[32:64, sl])
                nc.vector.tensor_mul(
                    dst[32:64, dsl], straight[0:32, :], sintS[0:32, sl])
                nc.vector.tensor_mul(
                    dst[64:96, dsl], straight[96:128, :], sintS[96:128, sl])
                nc.vector.tensor_mul(
                    dst[96:128, dsl], straight[64:96, :], sintS[64:96, sl])
                nc.vector.tensor_mul(straight[:], straight[:], cost[:, sl])
                nc.vector.tensor_add(dst[:, dsl], dst[:, dsl], straight[:])

            def emit_k_steps(jk, ib):
                ps = psum_mm.tile([P, NB], F32, tag="mm", name=f"psk{jk}{ib}")
                for c in range(KC):
                    nc.tensor.matmul(
                        ps[:], wkg[jk][:, c * P:(c + 1) * P], xchunk(c, ib),
                        start=(c == 0), stop=(c == KC - 1))
                    yield
                kr = spool.tile([P, NB], BF, tag="ropek")
                rope(ps, ib, kr, slice(0, NB))
                sl = slice(ib * NB, (ib + 1) * NB)
                for half in range(2):     # kv head 2jk+half, duplicated
                    src = kr[64 * half:64 * half + 64, :]
                    nc.gpsimd.tensor_copy(kdup[2 * jk + half][0:64, sl], src)
                    nc.gpsimd.tensor_copy(kdup[2 * jk + half][64:128, sl], src)
                yield

            def emit_v(ic):
                ps = psum_mm.tile([P, KCOLS], F32, tag="mm")
                ib, icc = ic // 4, ic % 4
                for c in range(KC):
                    nc.tensor.matmul(
                        ps[:], xchunk(c, ib)[:, icc * P:(icc + 1) * P],
                        wvt[c][:], start=(c == 0), stop=(c == KC - 1))
                for kv in range(KVL):
                    nc.scalar.copy(vaug[kv][ic][:, 0:D],
                                   ps[:, kv * D:(kv + 1) * D])

            def proj_q_ib_steps(jq, qr, ib):
                """Per-matmul steps of one 512-col block of chunk jq's q
                projection, rope emitted at the end.  The wq pair tile is
                looked up lazily so its DMA can be prefetched separately."""
                off = (jq % 2) * P
                wqg = wq_tiles[jq // 2]
                ps = psum_mm.tile([P, NB], F32, tag="mm", name=f"psq{jq}{ib}")
                for c in range(KC):
                    nc.tensor.matmul(
                        ps[:], wqg[:, c * 256 + off:c * 256 + off + P],
                        xchunk(c, ib), start=(c == 0), stop=(c == KC - 1))
                    yield
                rope(ps, ib, qr, slice(ib * NB, (ib + 1) * NB))
                yield

            class FillerQueue:
                """Ordered queue of lazily-created generators; attention
                pulls PE work from the front, drain points force specific
                entries to finish before dependent attention blocks."""

                def __init__(self):
                    self.entries = []       # [tag, iterator-or-thunk]
                    self.done = set()

                def add(self, tag, thunk):
                    self.entries.append([tag, thunk])

                def _iter(self, ent):
                    if callable(ent[1]):
                        ent[1] = ent[1]()
                    return ent[1]

                def _pop(self):
                    self.done.add(self.entries[0][0])
                    self.entries.pop(0)

                def fill(self, n):
                    for _ in range(n):
                        while self.entries:
                            try:
                                next(self._iter(self.entries[0]))
                                break
                            except StopIteration:
                                self._pop()
                        else:
                            return

                def drain(self, tag):
                    while tag not in self.done and self.entries:
                        ent = self.entries[0]
                        for _ in self._iter(ent):
                            pass
                        self._pop()

            def attention_qb(jq, qr, qb, queue):
                """Attention for the two heads of q-chunk jq, query block
                qb; PE work is drip-fed from `queue` between steps."""
                kvh = jq // 2
                nkj = 4 * (qb + 1)
                base = nkj - 4
                # diagonal chunks first (c=0..3), then full chunks
                seq = [(base + c, c) for c in range(4)] + \
                      [(kj, None) for kj in range(base)]
                oa = [psum_oa.tile([P, NB], F32, tag="oa", name=f"oa{p}")
                      for p in range(2)]

                def emit_oa(prev, step):
                    for p in range(2):
                        E, pkj, off, w = prev[p]
                        nc.tensor.matmul(
                            oa[p][:, off:NB], vaug[kvh][pkj][:],
                            E[:, 0:w], start=(step == 0),
                            stop=(step == len(seq) - 1))

                # software-pipelined: the oa pair for step s-1 is emitted
                # while step s's scores run, hiding exp(+mask) latency
                prev = None
                for step, (kj, c) in enumerate(seq):
                    off = P * c if c else 0
                    w = NB - off
                    cur = {}
                    for p in range(2):
                        hsl = slice(64 * p, 64 * p + 64)
                        sps = psum_sc.tile([P, NB], F32, tag="sc")
                        nc.tensor.matmul(
                            sps[:, 0:w],
                            kdup[kvh][hsl, kj * P:(kj + 1) * P],
                            qr[hsl, qb * NB + off:(qb + 1) * NB],
                            start=True, stop=True)
                        E = epool.tile([P, NB], BF, tag="E")
                        nc.scalar.activation(E[:, 0:w], sps[:, 0:w], Exp)
                        if c is not None:
                            eng = nc.gpsimd if p == 0 else nc.vector
                            eng.tensor_mul(E[:, 0:P], E[:, 0:P], maskt[:])
                        cur[p] = (E, kj, off, w)
                    queue.fill(2)
                    if prev is not None:
                        emit_oa(prev, step - 1)
                        queue.fill(1)
                    prev = cur
                queue.fill(10)
                emit_oa(prev, len(seq) - 1)
                qsl = slice(qb * NB, (qb + 1) * NB)
                for p in range(2):
                    rec = rpool.tile([64, NB], F32, tag="rec")
                    nc.vector.reciprocal(rec[:], oa[p][64:128, :])
                    nc.vector.tensor_mul(
                        aot[jq][64 * p:64 * p + 64, qsl],
                        oa[p][0:64, :], rec[:])

            wo_pairs = {}

            def load_wo_pair(pair):
                wog = wop.tile([P, 8 * 256], BF, tag="wopair")
                nc.sync.dma_start(
                    wog[:], woH[:, pair * 8 * 256:(pair + 1) * 8 * 256])
                return wog

            def finish_out_chunk(n, ib, fps, act=None):
                osb = outp.tile([P, NB], F32, tag="osb")
                if act if act is not None else (n + ib) % 2 == 0:
                    nc.scalar.copy(osb[:], fps[:])
                else:
                    nc.vector.tensor_copy(osb[:], fps[:])
                nc.sync.dma_start(
                    outT[n * P:(n + 1) * P, ib * NB:(ib + 1) * NB], osb[:])

            # out-proj chains opened during the (filler-less) last attention
            # block: hd 0..6 accumulate there, hd 7 completes afterwards.
            opened = []

            def outproj_partial_steps():
                for n, ib in ((0, 0), (0, 1), (1, 0)):
                    wog = wo_pairs[0]
                    off = (n % 2) * P
                    fps = psum_mm.tile([P, NB], F32, tag="mm",
                                       name=f"fps_pre{n}_{ib}")
                    opened.append((n, ib, fps))
                    for hd in range(7):
                        nc.tensor.matmul(
                            fps[:], wog[:, hd * 256 + off:hd * 256 + off + P],
                            aot[hd][:, ib * NB:(ib + 1) * NB],
                            start=(hd == 0), stop=False)
                        yield

            # ---- pre-attention: K, V, and the first q chunk ----
            # K chain pairs are interleaved 4 chunks at a time so the PE
            # consumes two matmuls per arriving x chunk during the x DMA;
            # the ib1 pair is woven between the V chains that pace on the
            # second-half x arrivals.
            def interleave4_steps(ga, gb):
                its = [ga, gb]
                live = [True, True]
                while any(live):
                    for i, g in enumerate(its):
                        if not live[i]:
                            continue
                        for _ in range(4):
                            try:
                                next(g)
                            except StopIteration:
                                live[i] = False
                                break
                            yield

            def pull(g, n):
                for _ in range(n):
                    try:
                        next(g)
                    except StopIteration:
                        return

            qrs = [qrtp.tile([P, S], BF, tag="qr", name=f"qr{jq}")
                   for jq in range(8)]
            for _ in interleave4_steps(emit_k_steps(0, 0),
                                       emit_k_steps(1, 0)):
                pass
            kpair2 = interleave4_steps(emit_k_steps(0, 1),
                                       emit_k_steps(1, 1))
            for ic in range(4):
                emit_v(ic)
                pull(kpair2, 9)
            for _ in kpair2:
                pass
            for ic in range(4, 8):
                emit_v(ic)
            for b in range(2):
                for _ in proj_q_ib_steps(0, qrs[0], b):
                    pass

            # ---- attention per q chunk, next chunk's projection drip-fed
            for jq in range(8):
                nxt = jq + 1
                queue = FillerQueue()
                if nxt < 8:
                    pr = nxt // 2
                    # prefetch the following pair one attention block early
                    if nxt % 2 == 1 and pr + 1 < 4 and pr + 1 not in wq_tiles:
                        wq_tiles[pr + 1] = load_wq_pair(pr + 1)
                    for b in range(2):
                        queue.add(f'p{nxt}{b}',
                                  (lambda b=b:
                                   proj_q_ib_steps(nxt, qrs[nxt], b)))
                else:
                    queue.add('op', outproj_partial_steps)
                if jq == 5:
                    wo_pairs[0] = load_wo_pair(0)
                    wo_pairs[1] = load_wo_pair(1)
                attention_qb(jq, qrs[jq], 0, queue)
                attention_qb(jq, qrs[jq], 1, queue)
                while queue.entries:    # leftover steps
                    queue.fill(1000)

            # ---- output projection ----
            for n, ib, fps in opened:
                wog = wo_pairs[0]
                off = (n % 2) * P
                nc.tensor.matmul(
                    fps[:], wog[:, 7 * 256 + off:7 * 256 + off + P],
                    aot[7][:, ib * NB:(ib + 1) * NB], start=False, stop=True)
                finish_out_chunk(n, ib, fps, act=True)
            done = {(n, ib) for n, ib, _ in opened}
            for n in range(DIM // P):
                pair = n // 2
                if n % 2 == 0 and pair + 1 < 8 and pair + 1 not in wo_pairs:
                    wo_pairs[pair + 1] = load_wo_pair(pair + 1)
                wog = wo_pairs[pair]
                off = (n % 2) * P
                for ib in range(2):
                    if (n, ib) in done:
                        continue
                    fps = psum_mm.tile([P, NB], F32, tag="mm")
                    for hd in range(8):
                        nc.tensor.matmul(
                            fps[:], wog[:, hd * 256 + off:hd * 256 + off + P],
                            aot[hd][:, ib * NB:(ib + 1) * NB],
                            start=(hd == 0), stop=(hd == 7))
                    finish_out_chunk(n, ib, fps)

    nc.compile()
    return nc


def host_inputs(x, freqs_cos, freqs_sin, wq, wk, wv, wo):
    """Build the 8 per-core input maps, pre-packed into SBUF tile layout."""
    import ml_dtypes
    bf16 = ml_dtypes.bfloat16

    x = np.asarray(x, np.float32)
    cos = np.asarray(freqs_cos, np.float32)
    sin = np.asarray(freqs_sin, np.float32)
    wq = np.asarray(wq, np.float32)
    wk = np.asarray(wk, np.float32)
    wv = np.asarray(wv, np.float32)
    wo = np.asarray(wo, np.float32)

    # de-interleave head dims: [t0 of 32 pairs | t1 of 32 pairs] per head,
    # so the rope pair-swap is a 32-partition half-swap per 64-row head.
    perm = np.concatenate([np.arange(0, D, 2), np.arange(1, D, 2)])

    # cos/sin tiles matching that row layout, [128, S] (two 64-row heads)
    cc = cos.T  # [32 pairs, S]
    ss = sin.T
    cos64 = np.concatenate([cc, cc], 0)
    # sinS is indexed by the SOURCE row of the swap: straight[t0 rows]
    # contributes +sin to the t1 rows, straight[t1 rows] -sin to t0 rows.
    sinS64 = np.concatenate([ss, -ss], 0)
    cosPa = np.ascontiguousarray(
        np.concatenate([cos64, cos64], 0)).astype(bf16)
    sinSPa = np.ascontiguousarray(
        np.concatenate([sinS64, sinS64], 0)).astype(bf16)

    # lower-triangle [128,128] mask (key j visible to query i iff j <= i)
    j = np.arange(P)[:, None]
    i = np.arange(P)[None, :]
    maskPa = np.ascontiguousarray((j <= i).astype(np.float32)).astype(bf16)

    scale = np.float32(1.0 / np.sqrt(D))
    in_maps = []
    for core in range(8):
        b, g = core // 2, core % 2

        # x: [DIM, S] -> [p, ib, c(16), e(512)]  (chunk-of-4 grouping is a
        # view detail on the SBUF side; DRAM layout is c-major per half)
        xT = x[b].T  # [2048, 1024]
        xHa = xT.reshape(KC, P, 2, NB).transpose(1, 2, 0, 3).reshape(P, -1)

        wq_g = wq[:, g * QCOLS:(g + 1) * QCOLS].reshape(DIM, HL, D)
        wq_g = (wq_g[:, :, perm] * scale).reshape(DIM, QCOLS)
        # wq: [DIM, 1024] -> [p, pair(4), c(16), e(256)]
        wqHa = wq_g.reshape(KC, P, 4, 256).transpose(1, 2, 0, 3).reshape(P, -1)

        wk_g = wk[:, g * KCOLS:(g + 1) * KCOLS].reshape(DIM, KVL, D)
        wk_g = wk_g[:, :, perm].reshape(DIM, KCOLS)
        # wk: [DIM, 256] -> [p, jk(2), c(16), e(128)]
        wkHa = wk_g.reshape(KC, P, 2, P).transpose(1, 2, 0, 3).reshape(P, -1)

        wv_g = wv[:, g * KCOLS:(g + 1) * KCOLS]
        # wv: [DIM, 256] -> [p, c(16), e(256)]
        wvHa = wv_g.reshape(KC, P, KCOLS).transpose(1, 0, 2).reshape(P, -1)

        wo_g = wo[g * QCOLS:(g + 1) * QCOLS, :]
        # wo: [1024, 2048] -> [p, pair(8), hd(8), e(256)]
        woHa = wo_g.reshape(8, P, 8, 256).transpose(1, 2, 0, 3).reshape(P, -1)

        in_maps.append({
            "xH": np.ascontiguousarray(xHa).astype(bf16),
            "wqH": np.ascontiguousarray(wqHa).astype(bf16),
            "wkH": np.ascontiguousarray(wkHa).astype(bf16),
            "wvH": np.ascontiguousarray(wvHa).astype(bf16),
            "woH": np.ascontiguousarray(woHa).astype(bf16),
            "cosP": cosPa,
            "sinSP": sinSPa,
            "maskP": maskPa,
        })
    return in_maps


_PROGRAM = None


def kernel(x, freqs_cos, freqs_sin, wq, wk, wv, wo):
    global _PROGRAM
    if _PROGRAM is None:
        _PROGRAM = build_program()
    nc = _PROGRAM
    in_maps = host_inputs(x, freqs_cos, freqs_sin, wq, wk, wv, wo)
    trace = os.environ.get("KERNEL_TRACE", "") == "1"
    if not trace:
        # the axon build here lacks the NTFF profile hook; make sure an
        # ambient BASS_TRACE can't route us into that (crashing) path
        os.environ["BASS_NEVER_TRACE"] = "1"
    res = run_bass_kernel_spmd(nc, in_maps, core_ids=list(range(8)),
                               trace=trace)
    if trace and res.exec_time_ns is not None:
        print(f"HW exec time: {res.exec_time_ns} ns")
        print(f"mean exec time: {res.mean_exec_time_ns} ns")
        if res.instructions_and_trace is not None:
            print("trace:", res.instructions_and_trace[1])
    out = np.zeros((B, S, DIM), np.float32)
    for core in range(8):
        b = core // 2
        out[b] += res.results[core]["outT"].T
    return out
